# revision 10
# baseline (speedup 1.0000x reference)
"""Trainium2 Bass kernel for nn_Conv1dAttention.

Math (per sample):
  q,k,v,pe = lrelu(bn(conv1d(x, W_p)))           # [C=128, L=2048], Cin=64, K=3
  S = q^T k                                      # [L, L]
  P = softmax_rows(S)                            # softmax over last axis
  out = v @ P + pe                               # [C, L]

Sharding: data-parallel over batch B=16 across 8 NeuronCores (2 samples/core).
Same NEFF on all cores, per-core input shards, no collectives.

Design notes (v3 — dense exp stream):
  - ScalarE does NOTHING but the 64 exp ACTIVATEs (the ~73.4us floor).  All
    conv drains run on DVE; softmax row-sums Z come free via the ACTIVATE
    accum_out port, killing the old per-block DVE z pass and shortening the
    exp->vts->out latency chain to ~0.4us.
  - PSUM: psA ring 2x[128,1024] (S blocks + conv fillers) + psO [128,2048]
    out accumulator = exactly 8 banks.  Fillers are [128,512]-granular units
    inserted in PAIRS between blocks so the S allocations keep their slot
    parity (S(b+1,h0) always lands on the slot exp(b,h0) just freed).
  - A filler's PSUM slot is released by the TS bias pass alone (the STT max
    pass reads the SBUF scratch), so slot turnaround is ~1us.
  - Prelude: xs0 loaded in 2 halves (1-col overlap so each half is
    self-contained for the conv taps), first exp needs only q0 quarter0 +
    k0 cols 0:1024 -> first ACTIVATE at ~12us instead of ~27us.
  - v^T bias via ONE ones-row matmul per 512-col group (bv pre-tiled 4x in
    the weight pack) instead of one per 128-col block.
  - Weights arrive in 2 packed DMAs: w12ext carries the k0/k1-tap weights
    for all 4 convs plus the fp32 conv biases bitcast into bf16 columns;
    w3ext carries the k2-tap weights plus the tiled v-bias row.
  - out chase lags 1 block; the final block's out-matmuls interleave with
    the finish add+store per 512-col chunk.
"""

import sys

if "/opt/trn_rl_repo" not in sys.path:
    sys.path.insert(0, "/opt/trn_rl_repo")

from contextlib import ExitStack

import ml_dtypes
import numpy as np

import concourse.bass as bass
import concourse.tile as tile
from concourse import bacc, mybir
from concourse.bass_utils import run_bass_kernel_spmd

B, CIN, COUT, KW, L = 16, 64, 128, 3, 2048
NCORES = 8
BP = B // NCORES  # samples per core
EPS = 1e-5
SLOPE = 0.3
F32 = mybir.dt.float32
BF16 = mybir.dt.bfloat16
NB = L // 128  # 16 attention blocks
HALF = 1024

_CACHE = {}

PCOL = {"q": 0, "k": 1, "v": 2, "p": 3}


def _body(ctx, tc, x, w12e, w3e, out):
    nc = tc.nc
    amax = mybir.AluOpType.max
    mult = mybir.AluOpType.mult
    aadd = mybir.AluOpType.add
    Exp = mybir.ActivationFunctionType.Exp

    wpool = ctx.enter_context(tc.tile_pool(name="wpool", bufs=1))
    apool = ctx.enter_context(tc.tile_pool(name="apool", bufs=2))
    ppool = ctx.enter_context(tc.tile_pool(name="ppool", bufs=4))
    vpool = ctx.enter_context(tc.tile_pool(name="vpool", bufs=3))
    zpool = ctx.enter_context(tc.tile_pool(name="zpool", bufs=3))
    lpool = ctx.enter_context(tc.tile_pool(name="lpool", bufs=3))
    opool = ctx.enter_context(tc.tile_pool(name="opool", bufs=4))
    psA = ctx.enter_context(tc.tile_pool(name="psA", bufs=2, space="PSUM"))
    psO = ctx.enter_context(tc.tile_pool(name="psO", bufs=1, space="PSUM"))

    # --- persistent tiles
    # w12ext layout: [:, 0:512] k0/k1-tap weights (q,k,v,p); [:, 512:520] the
    # fp32 conv biases bitcast to bf16 pairs; [0, 520:1032] v-bias tiled 4x.
    w12_t = wpool.tile([128, 4 * COUT + 8 + 512], BF16, tag="w12", name="w12")
    w3_t = wpool.tile([CIN, 4 * COUT], BF16, tag="w3", name="w3")
    ones1 = wpool.tile([1, COUT], BF16, tag="ones1", name="ones1")
    wseed = wpool.tile([128, 128], BF16, tag="wseed", name="wseed")
    xs0 = wpool.tile([128, L + 1], BF16, tag="xs0", name="xs0")
    xs1 = wpool.tile([128, L + 1], BF16, tag="xs1", name="xs1")
    nc.gpsimd.memset(ones1[:, :], 1.0)
    nc.gpsimd.memset(wseed[:, :], 0.001)
    nc.gpsimd.memset(xs0[0:CIN, L : L + 1], 0.0)
    nc.gpsimd.memset(xs0[CIN:128, 0:1], 0.0)
    nc.gpsimd.memset(xs1[0:CIN, L : L + 1], 0.0)
    nc.gpsimd.memset(xs1[CIN:128, 0:1], 0.0)

    # conv biases: fp32 values bitcast into 8 bf16 cols of w12ext
    bc_ap = w12_t[:, 4 * COUT : 4 * COUT + 8].bitcast(F32)  # [128, 4] f32
    bv4row = w12_t[0:1, 4 * COUT + 8 : 4 * COUT + 8 + 512]  # v-bias tiled 4x

    # --- DMA issue order: xs0 half0 first (gates the first exp), weights on
    # the scalar queue in parallel, then xs0 half1, then xs1.
    nc.sync.dma_start(out=xs0[0:CIN, 0 : HALF + 1], in_=x[0, :, 0 : HALF + 1])
    nc.sync.dma_start(out=xs0[CIN:128, 1 : HALF + 1], in_=x[0, :, 0:HALF])
    nc.scalar.dma_start(out=w12_t[:, :], in_=w12e[:, :])
    nc.scalar.dma_start(out=w3_t[:, :], in_=w3e[:, :])
    nc.sync.dma_start(out=xs0[0:CIN, HALF + 1 : L], in_=x[0, :, HALF + 1 : L])
    nc.sync.dma_start(out=xs0[CIN:128, HALF + 1 : L + 1], in_=x[0, :, HALF:L])
    nc.sync.dma_start(out=xs1[0:CIN, 0:L], in_=x[1, :, :])
    nc.sync.dma_start(out=xs1[CIN:128, 1 : L + 1], in_=x[1, :, :])

    def w12s(p):
        return w12_t[:, PCOL[p] * COUT : (PCOL[p] + 1) * COUT]

    def w3s(p):
        return w3_t[:, PCOL[p] * COUT : (PCOL[p] + 1) * COUT]

    n_psa = [0]  # psA allocation counter (slot parity bookkeeping)

    def psa_tile(w, name):
        n_psa[0] += 1
        return psA.tile([128, w], F32, tag="ps", name=name)

    def drain(dst_ap, ps_ap, p):
        # lrelu(y + b): TS bias-add psum->bf16 (releases the psum slot),
        # then STT max on the SBUF scratch.
        w = ps_ap.free_size()
        yb = lpool.tile([128, w], BF16, tag="yb", name="yb")
        if p is None:
            nc.vector.tensor_scalar_mul(yb[:, :], ps_ap, 1.0)
        else:
            nc.vector.tensor_scalar_add(yb[:, :], ps_ap, bc_ap[:, PCOL[p] : PCOL[p] + 1])
        nc.vector.scalar_tensor_tensor(
            dst_ap, yb[:, :], SLOPE, yb[:, :], op0=mult, op1=amax
        )

    def conv_q(xs, p, dst, q):
        # one [128,512] quarter of a [c,l]-layout conv: 1 psA alloc, 2 MMs
        cps = psa_tile(512, "cps")
        c0 = q * 512
        nc.tensor.matmul(cps[:, :], w12s(p), xs[:, c0 : c0 + 512], start=True, stop=False)
        nc.tensor.matmul(
            cps[:, :], w3s(p), xs[0:CIN, c0 + 1 : c0 + 513], start=False, stop=True
        )
        drain(dst[:, c0 : c0 + 512], cps[:, :], p)

    def vt_group(xs, vt, gh):
        # 4 l-blocks of V in transposed [l,c] layout + ONE bias matmul.
        vps = psa_tile(512, "vps")
        for i in range(4):
            blk = gh * 4 + i
            c = blk * 128
            pc = slice(i * 128, i * 128 + 128)
            # start=True only on i==0: start clears has_written for the whole
            # PSUM bank, which would make the single trailing bias matmul
            # overwrite (not accumulate onto) the earlier sub-blocks.
            nc.tensor.matmul(
                vps[:, pc], xs[:, c : c + 128], w12s("v"), start=(i == 0), stop=False
            )
            nc.tensor.matmul(
                vps[:, pc], xs[0:CIN, c + 1 : c + 129], w3s("v"), start=False, stop=False
            )
        nc.tensor.matmul(vps[:, :], ones1[0:1, :], bv4row, start=False, stop=True)
        drain(vt[:, gh * 512 : (gh + 1) * 512], vps[:, :], None)

    def make_tiles(s):
        q_t = apool.tile([128, L], BF16, tag="actq", name=f"q{s}")
        k_t = apool.tile([128, L], BF16, tag="actk", name=f"k{s}")
        pe_t = apool.tile([128, L], BF16, tag="actp", name=f"pe{s}")
        vt = apool.tile([128, L], BF16, tag="vt", name=f"vt{s}")
        return q_t, k_t, pe_t, vt

    def s_half(tiles, pblk, blk, h):
        # S matmuls + exp (with accumulated row-sum) for one [128,1024] half.
        q_t, k_t = tiles[0], tiles[1]
        sps = psa_tile(HALF, "sps")
        for n in range(2):
            c0 = h * HALF + n * 512
            nc.tensor.matmul(
                sps[:, n * 512 : n * 512 + 512],
                q_t[:, blk * 128 : blk * 128 + 128],
                k_t[:, c0 : c0 + 512],
                start=True,
                stop=True,
            )
        zh = zpool.tile([128, 1], F32, tag=f"z{h}", name=f"z{h}")
        nc.scalar.activation(
            pblk[:, h * HALF : (h + 1) * HALF], sps[:, :], Exp, accum_out=zh[:, :]
        )
        return zh

    def zfinish(tiles, blk, z0, z1):
        vt = tiles[3]
        zs = zpool.tile([128, 1], F32, tag="zs", name="zs")
        nc.vector.tensor_tensor(zs[:, :], z0[:, :], z1[:, :], aadd)
        r = zpool.tile([128, 1], F32, tag="r", name="r")
        nc.vector.reciprocal(r[:, :], zs[:, :])
        vts = vpool.tile([128, 128], BF16, tag="vts", name="vts")
        nc.vector.tensor_scalar_mul(vts[:, :], vt[:, blk * 128 : blk * 128 + 128], r[:, :])
        return vts

    def out_mms(out_ps, pblk, vts, blk, finish=None):
        for n in range(4):
            nc.tensor.matmul(
                out_ps[:, n * 512 : n * 512 + 512],
                vts[:, :],
                pblk[:, n * 512 : n * 512 + 512],
                start=(blk == 0),
                stop=(blk == NB - 1),
            )
            if finish is not None:
                finish(n)

    def finish_chunk(pe_t, out_ps, s, n):
        outs = opool.tile([128, 512], BF16, tag="outs", name=f"outc{n}")
        cols = slice(n * 512, (n + 1) * 512)
        nc.vector.tensor_tensor(outs[:, :], out_ps[:, cols], pe_t[:, cols], aadd)
        nc.sync.dma_start(out=out[s, :, cols], in_=outs[:, :])

    # --- PE warm-up through the DMA wait
    wps = psa_tile(128, "wps")
    for _ in range(16):
        nc.tensor.matmul(wps[:, :], wseed[:, :], wseed[:, :], start=True, stop=True)
    n_psa[0] = 0  # restart parity count; warmup slot is slot0

    tiles0 = make_tiles(0)
    q0, k0, pe0, vt0 = tiles0
    tiles1 = make_tiles(1)
    q1, k1, pe1, vt1 = tiles1

    # --- prelude: exactly what exp(b0) needs, nothing else
    conv_q(xs0, "q", q0, 0)   # psA #1 -> slot1
    conv_q(xs0, "k", k0, 0)   # #2 -> slot0
    conv_q(xs0, "k", k0, 1)   # #3 -> slot1

    pblk0 = ppool.tile([128, L], BF16, tag="pblk", name="pblk0")
    zh0 = s_half(tiles0, pblk0, 0, 0)          # #4 -> slot0
    conv_q(xs0, "k", k0, 2)   # #5 -> slot1
    conv_q(xs0, "k", k0, 3)   # #6 -> slot0
    zh1 = s_half(tiles0, pblk0, 0, 1)          # #7 -> slot1
    # vt0 g0 must precede zfinish(b0) in DVE program order; paired with q0q1
    # to keep the psA slot parity (every inter-S insertion is even-sized).
    vt_group(xs0, vt0, 0)     # #8
    conv_q(xs0, "q", q0, 1)   # #9

    # filler schedule: pairs of [512]-granular psA units per block
    fillB = {
        1: [lambda: vt_group(xs0, vt0, 1), lambda: conv_q(xs0, "q", q0, 2)],
        2: [lambda: vt_group(xs0, vt0, 2), lambda: conv_q(xs0, "q", q0, 3)],
        3: [lambda: vt_group(xs0, vt0, 3), lambda: conv_q(xs1, "k", k1, 0)],
        4: [lambda: conv_q(xs1, "k", k1, 1), lambda: conv_q(xs1, "k", k1, 2)],
        5: [lambda: conv_q(xs1, "k", k1, 3), lambda: conv_q(xs1, "q", q1, 0)],
        6: [lambda: conv_q(xs1, "q", q1, 1), lambda: conv_q(xs1, "q", q1, 2)],
        7: [lambda: conv_q(xs1, "q", q1, 3), lambda: conv_q(xs0, "p", pe0, 0)],
        8: [lambda: conv_q(xs0, "p", pe0, 1), lambda: conv_q(xs0, "p", pe0, 2)],
        9: [lambda: conv_q(xs0, "p", pe0, 3), lambda: vt_group(xs1, vt1, 0)],
    }
    fillC = {
        1: [lambda: vt_group(xs1, vt1, 3), lambda: conv_q(xs1, "p", pe1, 0)],
        2: [lambda: conv_q(xs1, "p", pe1, 1), lambda: conv_q(xs1, "p", pe1, 2)],
        15: [lambda: conv_q(xs1, "p", pe1, 3)],  # after last S: parity free
    }

    def attention_phase(tiles, out_ps, fillers, first_z, first_pblk, carry):
        """Blocks 1..15 of one sample; block 0's S/exp already emitted.
        carry = cross-phase PE work (previous sample's trailing outs),
        drained one item per block.  Returns last block's (pend, pblk, vts)."""
        z0, z1 = first_z
        pblk_prev = first_pblk
        pend = []  # (pblk, vts, blk) awaiting out_mms, lag 2
        for blk in range(1, NB):
            pblk = ppool.tile([128, L], BF16, tag="pblk", name=f"pblk{blk}")
            za = s_half(tiles, pblk, blk, 0)
            zb = s_half(tiles, pblk, blk, 1)
            # finish the PREVIOUS block's softmax scale + queue its out mms
            vts_prev = zfinish(tiles, blk - 1, z0, z1)
            pend.append((pblk_prev, vts_prev, blk - 1))
            if carry:
                carry.pop(0)()
            if len(pend) > 1:
                p, v, bb = pend.pop(0)
                out_mms(out_ps, p, v, bb)
            for f in fillers.get(blk, []):
                f()
            z0, z1 = za, zb
            pblk_prev = pblk
        # last block: zfinish; pending outs emitted by caller
        vts_last = zfinish(tiles, NB - 1, z0, z1)
        return pend, pblk_prev, vts_last

    out_ps0 = psO.tile([128, L], F32, tag="ops", name="out_ps0")
    pend0, pblkL0, vtsL0 = attention_phase(
        tiles0, out_ps0, fillB, (zh0, zh1), pblk0, []
    )

    # --- phase C: sample 1's S/exp starts immediately; sample 0's trailing
    # outs + finish ride along as carry work.
    pblk0c = ppool.tile([128, L], BF16, tag="pblk", name="pblk0c")
    zh0c = s_half(tiles1, pblk0c, 0, 0)
    zh1c = s_half(tiles1, pblk0c, 0, 1)
    vt_group(xs1, vt1, 1)     # transition pair: vt1 for C blocks 4..11
    vt_group(xs1, vt1, 2)

    carry = []
    for p, v, bb in pend0:
        carry.append(lambda p=p, v=v, bb=bb: out_mms(out_ps0, p, v, bb))
    carry.append(
        lambda: out_mms(
            out_ps0, pblkL0, vtsL0, NB - 1,
            finish=lambda n: finish_chunk(pe0, out_ps0, 0, n),
        )
    )

    out_ps1 = psO.tile([128, L], F32, tag="ops", name="out_ps1")
    pend1, pblkL1, vtsL1 = attention_phase(
        tiles1, out_ps1, fillC, (zh0c, zh1c), pblk0c, carry
    )
    for p, v, bb in pend1:
        out_mms(out_ps1, p, v, bb)
    out_mms(
        out_ps1, pblkL1, vtsL1, NB - 1,
        finish=lambda n: finish_chunk(pe1, out_ps1, 1, n),
    )


def build():
    nc = bacc.Bacc("TRN2", target_bir_lowering=False, debug=False)
    x_d = nc.dram_tensor("x", [BP, CIN, L], BF16, kind="ExternalInput")
    w12_d = nc.dram_tensor(
        "w12e", [128, 4 * COUT + 8 + 512], BF16, kind="ExternalInput"
    )
    w3_d = nc.dram_tensor("w3e", [CIN, 4 * COUT], BF16, kind="ExternalInput")
    out_d = nc.dram_tensor("out", [BP, COUT, L], BF16, kind="ExternalOutput")

    with tile.TileContext(nc) as tc, ExitStack() as ctx:
        _body(ctx, tc, x_d.ap(), w12_d.ap(), w3_d.ap(), out_d.ap())
    nc.compile()
    return nc


def _fold_weights(w, b, gamma, beta, mean, var):
    """Fold BN affine (fixed mean/var) into conv weights; split by tap."""
    w = np.asarray(w, np.float64)
    scale = np.asarray(gamma, np.float64) / np.sqrt(np.asarray(var, np.float64) + EPS)
    shift = np.asarray(beta, np.float64) - np.asarray(mean, np.float64) * scale
    wf = w * scale[:, None, None]  # [COUT, CIN, K]
    bf = np.asarray(b, np.float64) * scale + shift
    w12 = np.empty((128, COUT), np.float32)
    w12[0:CIN] = wf[:, :, 1].T
    w12[CIN:128] = wf[:, :, 0].T
    w3 = np.ascontiguousarray(wf[:, :, 2].T.astype(np.float32))  # [CIN, COUT]
    return w12, w3, bf.astype(np.float32)


def _get_nc():
    if "nc" not in _CACHE:
        _CACHE["nc"] = build()
    return _CACHE["nc"]


def make_in_maps(inputs):
    bf = ml_dtypes.bfloat16
    x = np.ascontiguousarray(np.asarray(inputs["x"], np.float32).astype(bf))
    folded = {}
    for p in "qkvp":
        key = p if p != "p" else "pe"
        folded[p] = _fold_weights(
            inputs[f"{key}_w"],
            inputs[f"{key}_b"],
            inputs[f"{key}_gamma"],
            inputs[f"{key}_beta"],
            inputs[f"{key}_mean"],
            inputs[f"{key}_var"],
        )
    w12pack = np.concatenate([folded[p][0] for p in "qkvp"], axis=1).astype(bf)
    bcols = np.stack([folded[p][2] for p in "qkvp"], axis=1).astype(np.float32)
    w12e = np.zeros((128, 4 * COUT + 8 + 512), dtype=bf)
    w12e[:, 0 : 4 * COUT] = w12pack
    w12e[:, 4 * COUT : 4 * COUT + 8] = np.ascontiguousarray(bcols).view(bf)
    w12e[0, 4 * COUT + 8 :] = np.tile(folded["v"][2].astype(bf), 4)
    w3e = np.ascontiguousarray(
        np.concatenate([folded[p][1] for p in "qkvp"], axis=1).astype(bf)
    )
    in_maps = []
    for i in range(NCORES):
        m = {
            "x": np.ascontiguousarray(x[i * BP : (i + 1) * BP]),
            "w12e": w12e,
            "w3e": w3e,
        }
        in_maps.append(m)
    return in_maps


def kernel(**inputs):
    nc = _get_nc()
    in_maps = make_in_maps(inputs)
    res = run_bass_kernel_spmd(nc, in_maps, core_ids=list(range(NCORES)))
    out = np.concatenate([res.results[i]["out"] for i in range(NCORES)], axis=0)
    return out.astype(np.float32)


if __name__ == "__main__":
    rng = np.random.default_rng(0)
    ins = {"x": rng.standard_normal((B, CIN, L), dtype=np.float32)}
    for p in ("q", "k", "v", "pe"):
        ins[f"{p}_w"] = (rng.standard_normal((COUT, CIN, KW)) * 0.05).astype(np.float32)
        ins[f"{p}_b"] = (rng.standard_normal(COUT) * 0.05).astype(np.float32)
        ins[f"{p}_gamma"] = rng.uniform(0.5, 1.5, COUT).astype(np.float32)
        ins[f"{p}_beta"] = (rng.standard_normal(COUT) * 0.05).astype(np.float32)
        ins[f"{p}_mean"] = (rng.standard_normal(COUT) * 0.05).astype(np.float32)
        ins[f"{p}_var"] = rng.uniform(0.5, 1.5, COUT).astype(np.float32)
    got = kernel(**ins)
    print("kernel output:", got.shape, got.dtype, np.abs(got).mean())
